# revision 4
# baseline (speedup 1.0000x reference)
"""Deformable conv v3: block-table gather + DVE/ACT split combine + DMA transpose.

Sharding: (batch=4) x (row-halves=2) -> 8 cores, 2048 positions/core.
Key changes vs baseline:
  - 2x2-corner block table xblk[4225, 1024]: ONE gather descriptor per
    (pos, tap) fetches all 4 bilinear corners (desc-count-bound gather).
  - corner combine: DVE does corners 0,1 (ts-mult + stt), ACT does 2,3
    (scale-copies), DVE wide tt-adds merge.
  - cols transposed to ch-major via batched DMA transpose on sync engine
    (out[c,j,p] = in[p,j,c]) -- no PE transposes, no psum copies.
  - main matmul accumulates per-tap into all 8 PSUM banks.
  - GN group reduce/broadcast via partition-crossing DMAs (no PSUM needed);
    one merged [16,4] pair-AllReduce.
"""
import sys

sys.path.insert(0, "/opt/trn_rl_repo/concourse")
sys.path.insert(0, "/opt/trn_rl_repo")

import numpy as np
import ml_dtypes

import concourse.bass as bass
import concourse.bacc as bacc
import concourse.tile as tile
import concourse.mybir as mybir
from concourse.bass_utils import run_bass_kernel_spmd

F32 = mybir.dt.float32
BF16 = mybir.dt.bfloat16
I16 = mybir.dt.int16
I32 = mybir.dt.int32
AOT = mybir.AluOpType
ACTF = mybir.ActivationFunctionType

B, C, H, W = 4, 256, 64, 64
P9 = 9
EPS = 1e-5
N_CORES = 8
NJ = 16
SH = 16.0


def _ident(nc, name, dt):
    from concourse.masks import make_identity
    t = nc.alloc_sbuf_tensor(name, [128, 128], dt)
    make_identity(nc, t.ap())
    return t.ap()


def build_nc(with_collective=True):
    nc = bacc.Bacc("TRN2", target_bir_lowering=False, debug=False,
                   num_devices=N_CORES if with_collective else 1)
    DIV = 32768.0 if with_collective else 16384.0

    # ---------------- I/O ----------------
    xblk = nc.dram_tensor("xblk", [4225, 1024], BF16, kind="ExternalInput")
    xconv = nc.dram_tensor("xconv", [2, 128, 34, 66], BF16, kind="ExternalInput").ap()
    wofft = nc.dram_tensor("wofft", [128, 2, 9, 18], BF16, kind="ExternalInput").ap()
    wt = nc.dram_tensor("wt", [128, 2, 9, 256], BF16, kind="ExternalInput").ap()
    cy = nc.dram_tensor("cy", [128, 16, 9], F32, kind="ExternalInput").ap()
    cx = nc.dram_tensor("cx", [128, 16, 9], F32, kind="ExternalInput").ap()
    bvec = nc.dram_tensor("bvec", [128, 2], F32, kind="ExternalInput").ap()
    gamT = nc.dram_tensor("gamT", [16, 8, 2], F32, kind="ExternalInput").ap()
    betT = nc.dram_tensor("betT", [16, 8, 2], F32, kind="ExternalInput").ap()
    bT = nc.dram_tensor("bT", [16, 8, 2], F32, kind="ExternalInput").ap()
    boff = nc.dram_tensor("boff", [18, 1], F32, kind="ExternalInput").ap()
    yout = nc.dram_tensor("yout", [2, 128, 2048], F32, kind="ExternalOutput").ap()

    with tile.TileContext(nc) as tc:
        # persistent sbuf tensors
        sb_wofft = nc.alloc_sbuf_tensor("sb_wofft", [128, 2, 9, 18], BF16).ap()
        sb_wt = nc.alloc_sbuf_tensor("sb_wt", [128, 2, 9, 256], BF16).ap()
        sb_cy = nc.alloc_sbuf_tensor("sb_cy", [128, 16, 9], F32).ap()
        sb_cx = nc.alloc_sbuf_tensor("sb_cx", [128, 16, 9], F32).ap()
        sb_bvec = nc.alloc_sbuf_tensor("sb_bvec", [128, 2], F32).ap()
        sb_gamT = nc.alloc_sbuf_tensor("sb_gamT", [16, 8, 2], F32).ap()
        sb_betT = nc.alloc_sbuf_tensor("sb_betT", [16, 8, 2], F32).ap()
        sb_bT = nc.alloc_sbuf_tensor("sb_bT", [16, 8, 2], F32).ap()
        sb_boff = nc.alloc_sbuf_tensor("sb_boff", [18, 1], F32).ap()
        offv = nc.alloc_sbuf_tensor("offv", [128, 16, 18], F32).ap()
        w4 = nc.alloc_sbuf_tensor("w4", [128, 16, 9, 4], F32).ap()
        idxs16 = nc.alloc_sbuf_tensor("idxs16", [128, 9, 128], I16)
        sums = nc.alloc_sbuf_tensor("sums", [128, 2, 4], F32).ap()
        sumsq = nc.alloc_sbuf_tensor("sumsq", [128, 2, 4], F32).ap()
        st2 = nc.alloc_sbuf_tensor("st2", [128, 2, 2], F32).ap()   # per o2
        st8 = nc.alloc_sbuf_tensor("st8", [16, 2, 8, 2], F32).ap()
        allst = nc.alloc_sbuf_tensor("allst", [16, 4], F32).ap()
        allst2 = nc.alloc_sbuf_tensor("allst2", [16, 4], F32).ap()
        mr = nc.alloc_sbuf_tensor("mr", [16, 2, 2], F32).ap()      # (mean,rstd) x o2
        scbn16 = nc.alloc_sbuf_tensor("scbn16", [16, 8, 2, 2], F32).ap()  # (i, kind, o2)
        scbn_pp = nc.alloc_sbuf_tensor("scbn_pp", [128, 2, 2], F32).ap()
        sb_eps = nc.alloc_sbuf_tensor("sb_eps", [16, 1], F32).ap()
        scr = nc.alloc_sbuf_tensor("scr", [128, 512], BF16).ap()
        scr2 = scr

        nc.vector.memset(sb_eps[:], EPS)
        id32 = _ident(nc, "id32", F32)

        # ---------------- loads ----------------
        xblk_ap = bass.AP(tensor=xblk, offset=0, ap=[[1024, 4225], [1, 1024]])

        with (
            tc.tile_pool(name="g", bufs=2) as gpool,
            tc.tile_pool(name="mth", bufs=18) as mth,
        ):
            sb_xconv = gpool.tile([128, 2, 34, 66], BF16, tag="g", name="sb_xconv")
            off_sb = gpool.tile([18, 2048], F32, tag="g", name="off_sb")
            nc.sync.dma_start(out=sb_xconv[:], in_=bass.AP(
                tensor=xconv.tensor, offset=0,
                ap=[[34 * 66, 128], [128 * 34 * 66, 2], [66, 34], [1, 66]]))
            nc.sync.dma_start(out=sb_wofft[:], in_=wofft)
            nc.sync.dma_start(out=sb_wt[:], in_=wt)
            nc.sync.dma_start(out=sb_cy[:], in_=cy)
            nc.sync.dma_start(out=sb_cx[:], in_=cx)
            nc.sync.dma_start(out=sb_bvec[:], in_=bvec)
            nc.sync.dma_start(out=sb_gamT[:], in_=gamT)
            nc.sync.dma_start(out=sb_betT[:], in_=betT)
            nc.sync.dma_start(out=sb_bT[:], in_=bT)
            nc.sync.dma_start(out=sb_boff[:], in_=boff)

            # ------------- phases 1-4, pipelined per position-half jh -------
            with tc.tile_pool(name="ps_pro", bufs=2, space="PSUM") as pss:
                _mtc = [0]

                def mt(shape=(128, 8, 9), dt=F32, tag="m"):
                    _mtc[0] += 1
                    return mth.tile(list(shape), dt, tag=tag, name=f"mt{_mtc[0]}",
                                    bufs=18 if tag == "m" else 4)

                for jh in range(2):
                    j8 = slice(jh * 8, (jh + 1) * 8)
                    # phase 1: offset conv for this half (rc chunks 2jh, 2jh+1)
                    for rc in (2 * jh, 2 * jh + 1):
                        ps = pss.tile([18, 512], F32, tag="ps", space="PSUM")
                        k = 0
                        for t in range(9):
                            for c2 in range(2):
                                ky, kx = t // 3, t % 3
                                rv = sb_xconv[:, c2, rc * 8 + ky: rc * 8 + ky + 8, kx: kx + 64]
                                nc.tensor.matmul(ps[:], lhsT=sb_wofft[:, c2, t, :], rhs=rv,
                                                 start=(k == 0), stop=(k == 17))
                                k += 1
                        nc.vector.tensor_scalar(out=off_sb[:, rc * 512:(rc + 1) * 512],
                                                in0=ps[:], scalar1=sb_boff[:, 0:1],
                                                scalar2=None, op0=AOT.add)
                    # phase 2: transpose offsets to [pos, 18]
                    for jj in range(jh * 8, (jh + 1) * 8):
                        pt = pss.tile([128, 18], F32, tag="ps", space="PSUM")
                        nc.tensor.transpose(pt[:], off_sb[:, jj * 128:(jj + 1) * 128],
                                            id32[:18, :18])
                        nc.vector.tensor_copy(out=offv[:, jj, :], in_=pt[:])

                    # phase 3: coords / weights / indices for this half
                    dy = offv[:, j8, 0:18:2]
                    dx = offv[:, j8, 1:18:2]
                    ys, xs = mt(), mt()
                    nc.vector.tensor_add(out=ys[:], in0=dy, in1=sb_cy[:, j8, :])
                    nc.vector.tensor_add(out=xs[:], in0=dx, in1=sb_cx[:, j8, :])
                    fy, fx, y0, x0 = mt(), mt(), mt(), mt()
                    # floor(v): i=round(v); floor = i - (i > v); frac = v - floor
                    for src_, fl_, fr_ in ((ys, y0, fy), (xs, x0, fx)):
                        ic = mt(dt=I32, tag="mi")
                        icf = mt()
                        gt_ = mt()
                        nc.vector.tensor_copy(out=ic[:], in_=src_[:])
                        nc.vector.tensor_copy(out=icf[:], in_=ic[:])
                        nc.vector.tensor_tensor(out=gt_[:], in0=icf[:], in1=src_[:], op=AOT.is_gt)
                        nc.vector.tensor_tensor(out=fl_[:], in0=icf[:], in1=gt_[:], op=AOT.subtract)
                        nc.vector.tensor_tensor(out=fr_[:], in0=src_[:], in1=fl_[:], op=AOT.subtract)
                    wy0, wx0 = mt(), mt()
                    nc.vector.tensor_scalar(out=wy0[:], in0=fy[:], scalar1=-1.0, scalar2=1.0,
                                            op0=AOT.mult, op1=AOT.add)
                    nc.vector.tensor_scalar(out=wx0[:], in0=fx[:], scalar1=-1.0, scalar2=1.0,
                                            op0=AOT.mult, op1=AOT.add)

                    def mask_in(src, lo, hi):
                        g_, l_, m_ = mt(), mt(), mt()
                        nc.vector.tensor_scalar(out=g_[:], in0=src[:], scalar1=lo, scalar2=None, op0=AOT.is_ge)
                        nc.vector.tensor_scalar(out=l_[:], in0=src[:], scalar1=hi, scalar2=None, op0=AOT.is_le)
                        nc.vector.tensor_tensor(out=m_[:], in0=g_[:], in1=l_[:], op=AOT.mult)
                        return m_

                    my0 = mask_in(y0, 16.0, 79.0)
                    my1 = mask_in(y0, 15.0, 78.0)
                    mx0 = mask_in(x0, 16.0, 79.0)
                    mx1 = mask_in(x0, 15.0, 78.0)

                    wy0e, wy1e, wx0e, wx1e = mt(), mt(), mt(), mt()
                    nc.vector.tensor_tensor(out=wy0e[:], in0=wy0[:], in1=my0[:], op=AOT.mult)
                    nc.vector.tensor_tensor(out=wy1e[:], in0=fy[:], in1=my1[:], op=AOT.mult)
                    nc.vector.tensor_tensor(out=wx0e[:], in0=wx0[:], in1=mx0[:], op=AOT.mult)
                    nc.vector.tensor_tensor(out=wx1e[:], in0=fx[:], in1=mx1[:], op=AOT.mult)

                    # w4 block-slot order: (y0,x0), (y0,x1), (y1,x0), (y1,x1)
                    nc.vector.tensor_tensor(out=w4[:, j8, :, 0], in0=wy0e[:], in1=wx0e[:], op=AOT.mult)
                    nc.vector.tensor_tensor(out=w4[:, j8, :, 1], in0=wy0e[:], in1=wx1e[:], op=AOT.mult)
                    nc.vector.tensor_tensor(out=w4[:, j8, :, 2], in0=wy1e[:], in1=wx0e[:], op=AOT.mult)
                    nc.vector.tensor_tensor(out=w4[:, j8, :, 3], in0=wy1e[:], in1=wx1e[:], op=AOT.mult)

                    # block idx: iy = clamp(y0s,15,79); idx = iy*65 + ix - 990
                    yc, xc, idxf = mt(), mt(), mt()
                    nc.vector.tensor_scalar(out=yc[:], in0=y0[:], scalar1=15.0, scalar2=79.0,
                                            op0=AOT.max, op1=AOT.min)
                    nc.vector.tensor_scalar(out=xc[:], in0=x0[:], scalar1=15.0, scalar2=79.0,
                                            op0=AOT.max, op1=AOT.min)
                    nc.vector.scalar_tensor_tensor(out=idxf[:], in0=yc[:], scalar=65.0,
                                                   in1=xc[:], op0=AOT.mult, op1=AOT.add)

                    # phase 4: idx transposes into wrapped layout
                    # idxs16[r, t, jj*8+q] = idxf[16q+r, jj-jh*8, t] - 990
                    idxt_t = mt(shape=(9, 8, 128), tag="it")
                    for jl in range(8):
                        pi = pss.tile([9, 128], F32, tag="ps", space="PSUM")
                        nc.tensor.transpose(pi[:], idxf[:, jl, :], id32[:, :])
                        nc.vector.tensor_copy(out=idxt_t[:, jl, :], in_=pi[:])
                    for jl in range(8):
                        jj = jh * 8 + jl
                        ptw = pss.tile([16, 8, 9], F32, tag="ps", space="PSUM")
                        for q in range(8):
                            nc.tensor.transpose(ptw[:, q, :],
                                                idxt_t[:, jl, 16 * q:16 * q + 16],
                                                id32[:9, :9])
                        nc.vector.tensor_scalar(
                            out=idxs16.ap()[0:16, :, jj * 8:(jj + 1) * 8],
                            in0=ptw[:].rearrange("r a t -> r t a"),
                            scalar1=-990.0, scalar2=None, op0=AOT.add)
                    for g8 in range(1, 8):
                        nc.sync.dma_start(
                            out=idxs16.ap()[g8 * 16:(g8 + 1) * 16, :, jh * 64:(jh + 1) * 64],
                            in_=idxs16.ap()[0:16, :, jh * 64:(jh + 1) * 64])

            # ---------------- phases 5-8: gather/combine/transpose/matmul ----
            with (
                tc.tile_pool(name="gb", bufs=6) as gbpool,
                tc.tile_pool(name="ac", bufs=4) as acpool,
                tc.tile_pool(name="tm", bufs=2) as tmpool,
                tc.tile_pool(name="rh", bufs=4) as rhpool,
                tc.tile_pool(name="psy", bufs=8, space="PSUM") as psy,
                tc.tile_pool(name="yo", bufs=1) as yopool,
                tc.tile_pool(name="dram", bufs=2, space="DRAM") as drpool,
            ):
                ps_y = []
                for o2 in range(2):
                    for nt in range(4):
                        ps_y.append(psy.tile([128, 512], F32, tag="y", space="PSUM",
                                             name=f"psy{o2}_{nt}"))
                gsems = [nc.alloc_semaphore(f"gsem{i}") for i in range(18)]
                if with_collective:
                    win_ = drpool.tile([16, 4], F32, tag="win")
                    wout = drpool.tile([16, 4], F32, tag="wout")
                    nc.sync.dma_start(out=win_[:], in_=allst[:])
                    nc.gpsimd.collective_compute(
                        "AllReduce", AOT.add,
                        replica_groups=[[0, 1], [2, 3], [4, 5], [6, 7]],
                        ins=[win_.opt()], outs=[wout.opt()])

                for t in range(9):
                    for jh in range(2):
                        g_ = gbpool.tile([128, 8, 1024], BF16, tag="gb")
                        nc.gpsimd.dma_gather(
                            out_ap=g_[:], in_ap=xblk_ap,
                            idxs_ap=idxs16.ap()[:, t, jh * 64:(jh + 1) * 64],
                            num_idxs=1024, num_idxs_reg=1024,
                            elem_size=1024, elem_step=1024,
                            prepare_only=True, sem=gsems[2 * t + jh])
                        nc.gpsimd.trigger_dma(count=1)
                        acc = acpool.tile([128, 2, 8, 128], BF16, tag="ac")
                        rhs_t = rhpool.tile([128, 2, 8, 128], BF16, tag="rh")
                        tA = tmpool.tile([128, 2, 2, 8, 128], BF16, tag="tm")
                        nc.vector.wait_ge(gsems[2 * t + jh], 16)
                        nc.scalar.wait_ge(gsems[2 * t + jh], 16)
                        for j in range(8):
                            jj = jh * 8 + j
                            gv = [g_[:, j, k * 256:(k + 1) * 256]
                                  .rearrange("p (a b) -> p a b", a=2) for k in range(4)]
                            sc = [w4[:, jj, t, k:k + 1] for k in range(4)]
                            # DVE: corners 0,1
                            nc.vector.tensor_scalar(out=acc[:, :, j, :], in0=gv[0],
                                                    scalar1=sc[0], scalar2=None, op0=AOT.mult)
                            nc.vector.scalar_tensor_tensor(out=acc[:, :, j, :], in0=gv[1],
                                                           scalar=sc[1], in1=acc[:, :, j, :],
                                                           op0=AOT.mult, op1=AOT.add)
                            # ACT: corners 2,3
                            nc.scalar.activation(out=tA[:, 0, :, j, :], in_=gv[2],
                                                 func=ACTF.Copy, scale=sc[2])
                            nc.scalar.activation(out=tA[:, 1, :, j, :], in_=gv[3],
                                                 func=ACTF.Copy, scale=sc[3])
                        # DVE wide adds
                        nc.vector.tensor_tensor(out=tA[:, 0], in0=tA[:, 0], in1=tA[:, 1],
                                                op=AOT.add)
                        nc.vector.tensor_tensor(out=acc[:], in0=acc[:], in1=tA[:, 0],
                                                op=AOT.add)
                        # DMA transpose to ch-major: rhs_t[c, j, p] = acc[p, j, c]
                        for c2 in range(2):
                            nc.sync.dma_start_transpose(
                                out=rhs_t[:, c2, :, :],
                                in_=acc[:, c2, :, :])
                        # main matmul accumulation for this half
                        for o2 in range(2):
                            for c2 in range(2):
                                lt = sb_wt[:, c2, t, o2 * 128:(o2 + 1) * 128]
                                for nl in range(2):
                                    nt = 2 * jh + nl
                                    nc.tensor.matmul(
                                        ps_y[o2 * 4 + nt][:], lhsT=lt,
                                        rhs=rhs_t[:, c2, nl * 4:(nl + 1) * 4, :]
                                        .rearrange("c a b -> c (a b)"),
                                        start=(t == 0 and c2 == 0),
                                        stop=(t == 8 and c2 == 1))
                        if t == 8:
                            for o2 in range(2):
                                for nl in range(2):
                                    nt = 2 * jh + nl
                                    nc.scalar.activation(
                                        out=scr[:], in_=ps_y[o2 * 4 + nt][:],
                                        func=ACTF.Copy,
                                        accum_out=sums[:, o2, nt:nt + 1])
                                    nc.scalar.activation(
                                        out=scr2[:], in_=ps_y[o2 * 4 + nt][:],
                                        func=ACTF.Square,
                                        accum_out=sumsq[:, o2, nt:nt + 1])

                # ---------------- phase 9: GN stats ----------------
                for o2 in range(2):
                    sy = mth.tile([128, 1], F32, tag="s1", bufs=12, name="sy")
                    qy = mth.tile([128, 1], F32, tag="s1", bufs=12)
                    t1_ = mth.tile([128, 1], F32, tag="s1", bufs=12)
                    t2_ = mth.tile([128, 1], F32, tag="s1", bufs=12)
                    t3_ = mth.tile([128, 1], F32, tag="s1", bufs=12)
                    nc.vector.reduce_sum(out=sy[:], in_=sums[:, o2, :],
                                         axis=mybir.AxisListType.X)
                    nc.vector.reduce_sum(out=qy[:], in_=sumsq[:, o2, :],
                                         axis=mybir.AxisListType.X)
                    b_ap = sb_bvec[:, o2:o2 + 1]
                    # Sz = Sy + 2048*b ; Qz = Qy + 2*b*Sy + 2048*b^2
                    nc.vector.scalar_tensor_tensor(out=st2[:, o2, 0:1], in0=b_ap,
                                                   scalar=2048.0, in1=sy[:],
                                                   op0=AOT.mult, op1=AOT.add)
                    nc.vector.scalar_tensor_tensor(out=t1_[:], in0=b_ap, scalar=2.0,
                                                   in1=sy[:], op0=AOT.mult, op1=AOT.mult)
                    nc.vector.scalar_tensor_tensor(out=t2_[:], in0=b_ap, scalar=2048.0,
                                                   in1=b_ap, op0=AOT.mult, op1=AOT.mult)
                    nc.vector.tensor_tensor(out=t3_[:], in0=t1_[:], in1=t2_[:], op=AOT.add)
                    nc.vector.tensor_tensor(out=st2[:, o2, 1:2], in0=qy[:], in1=t3_[:], op=AOT.add)
                    # partition fold 128 -> 16 groups x 8: st8[g, o2, i, c] = st2[8g+i, o2, c]
                    nc.sync.dma_start(out=st8[:, o2, :, :], in_=st2[:, o2, :])
                # tree-add over i: [16,2,8,2] -> allst [16,4]
                a4 = mth.tile([16, 2, 4, 2], F32, tag="a4", bufs=2)
                a2 = mth.tile([16, 2, 2, 2], F32, tag="a2", bufs=2)
                nc.vector.tensor_tensor(out=a4[:], in0=st8[:, :, 0:4, :],
                                        in1=st8[:, :, 4:8, :], op=AOT.add)
                nc.vector.tensor_tensor(out=a2[:], in0=a4[:, :, 0:2, :],
                                        in1=a4[:, :, 2:4, :], op=AOT.add)
                nc.vector.tensor_tensor(out=allst[:].rearrange("p (a b) -> p a b", a=2),
                                        in0=a2[:, :, 0, :], in1=a2[:, :, 1, :], op=AOT.add)

                # ---------------- phase 10: pair AllReduce ----------------
                if with_collective:
                    bin_ = drpool.tile([16, 4], F32, tag="cin")
                    bout = drpool.tile([16, 4], F32, tag="cout")
                    nc.sync.dma_start(out=bin_[:], in_=allst[:])
                    nc.gpsimd.collective_compute(
                        "AllReduce", AOT.add,
                        replica_groups=[[0, 1], [2, 3], [4, 5], [6, 7]],
                        ins=[bin_.opt()], outs=[bout.opt()])
                    nc.sync.dma_start(out=allst2[:], in_=bout[:])
                else:
                    nc.vector.tensor_copy(out=allst2[:], in_=allst[:])

                # ---------------- phase 11: mean/rstd, scale/bias, relu ------
                var = mth.tile([16, 2], F32, tag="v16", bufs=2)
                m2 = mth.tile([16, 2], F32, tag="v16", bufs=2)
                av2 = allst2.rearrange("p (a b) -> p a b", a=2)
                nc.vector.tensor_scalar(out=mr[:, :, 0], in0=av2[:, :, 0],
                                        scalar1=1.0 / DIV, scalar2=None, op0=AOT.mult)
                nc.vector.tensor_tensor(out=m2[:], in0=mr[:, :, 0], in1=mr[:, :, 0],
                                        op=AOT.mult)
                nc.vector.tensor_scalar(out=var[:], in0=av2[:, :, 1],
                                        scalar1=1.0 / DIV, scalar2=None, op0=AOT.mult)
                nc.vector.tensor_tensor(out=var[:], in0=var[:], in1=m2[:], op=AOT.subtract)
                nc.scalar.activation(out=var[:], in_=var[:], func=ACTF.Sqrt, bias=sb_eps[:])
                nc.vector.reciprocal(out=mr[:, :, 1], in_=var[:])
                # sc16 = gamT * rstd ; bn16 = (bT - mean)*sc16 + betT
                tb = mth.tile([16, 8, 2], F32, tag="tb", bufs=4)
                for o2 in range(2):
                    nc.vector.tensor_scalar(out=scbn16[:, :, 0, o2], in0=sb_gamT[:, :, o2],
                                            scalar1=mr[:, o2, 1:2], scalar2=None, op0=AOT.mult)
                    nc.vector.tensor_scalar(out=tb[:, :, o2], in0=sb_bT[:, :, o2],
                                            scalar1=mr[:, o2, 0:1], scalar2=None,
                                            op0=AOT.subtract)
                nc.vector.tensor_tensor(out=tb[:], in0=tb[:], in1=scbn16[:, :, 0, :], op=AOT.mult)
                nc.vector.tensor_tensor(out=scbn16[:, :, 1, :], in0=tb[:], in1=sb_betT[:], op=AOT.add)
                # partition-expand [16, 8, 2, 2] -> [128, 2, 2] in one DMA
                nc.sync.dma_start(out=scbn_pp[:], in_=scbn16[:])

                for o2 in range(2):
                    for np_ in range(2):
                        yo = yopool.tile([128, 2, 512], F32, tag="yo")
                        for nl in range(2):
                            nt = np_ * 2 + nl
                            if (nt + o2) % 2 == 0:
                                nc.scalar.activation(out=yo[:, nl, :], in_=ps_y[o2 * 4 + nt][:],
                                                     func=ACTF.Relu, scale=scbn_pp[:, 0, o2:o2 + 1],
                                                     bias=scbn_pp[:, 1, o2:o2 + 1])
                            else:
                                nc.vector.tensor_scalar(out=yo[:, nl, :], in0=ps_y[o2 * 4 + nt][:],
                                                        scalar1=scbn_pp[:, 0, o2:o2 + 1],
                                                        scalar2=scbn_pp[:, 1, o2:o2 + 1],
                                                        op0=AOT.mult, op1=AOT.add)
                                nc.vector.tensor_scalar(out=yo[:, nl, :], in0=yo[:, nl, :],
                                                        scalar1=0.0, scalar2=None,
                                                        op0=AOT.max)
                        nc.sync.dma_start(out=yout[o2, :, np_ * 1024:(np_ + 1) * 1024],
                                          in_=yo[:].rearrange("p a b -> p (a b)"))

    nc.compile()
    return nc


# ---------------------------------------------------------------------------
# host side
# ---------------------------------------------------------------------------
_NC_CACHE = {}


def get_nc(with_collective=True):
    key = with_collective
    if key not in _NC_CACHE:
        _NC_CACHE[key] = build_nc(with_collective)
    return _NC_CACHE[key]


def make_in_maps(x, w_off, b_off, w, b, gamma, beta):
    p = np.arange(128)
    j = np.arange(16)
    t = np.arange(9)
    cxv = (((p % 64)[:, None, None] + (t % 3)[None, None, :] - 1 + SH)
           + np.zeros((1, 16, 1))).astype(np.float32)
    # weight layouts
    w4d = w.reshape(256, 2, 128, 3, 3)
    wt_ = np.ascontiguousarray(
        w4d.reshape(256, 2, 128, 9).transpose(2, 1, 3, 0)).astype(ml_dtypes.bfloat16)
    wo4d = w_off.reshape(18, 2, 128, 9)
    wofft_ = np.ascontiguousarray(wo4d.transpose(2, 1, 3, 0)).astype(ml_dtypes.bfloat16)
    bvec_ = np.ascontiguousarray(b.reshape(2, 128).T).astype(np.float32)
    boff_ = b_off.reshape(18, 1).astype(np.float32)
    # [16 g, 2 o2, 8 i] layouts: value at channel o2*128 + 8g + i
    gamT_ = np.ascontiguousarray(gamma.reshape(2, 16, 8).transpose(1, 2, 0)).astype(np.float32)
    betT_ = np.ascontiguousarray(beta.reshape(2, 16, 8).transpose(1, 2, 0)).astype(np.float32)
    bT_ = np.ascontiguousarray(b.reshape(2, 16, 8).transpose(1, 2, 0)).astype(np.float32)

    in_maps = []
    xblk_cache = {}
    for core in range(N_CORES):
        bb, half = core // 2, core % 2
        base = 32 * half
        xb = x[bb]                                   # [256, 64, 64]
        if bb not in xblk_cache:
            xp = np.zeros((66, 66, 256), np.float32)
            xp[1:65, 1:65, :] = xb.transpose(1, 2, 0)
            blk = np.stack([xp[0:65, 0:65], xp[0:65, 1:66],
                            xp[1:66, 0:65], xp[1:66, 1:66]], axis=2)  # [65,65,4,256]
            xblk_cache[bb] = np.ascontiguousarray(
                blk.reshape(4225, 1024)).astype(ml_dtypes.bfloat16)
        xc = np.zeros((2, 128, 34, 66), ml_dtypes.bfloat16)
        r0, r1 = base - 1, base + 33
        cr0, cr1 = max(r0, 0), min(r1, 64)
        xc[:, :, cr0 - r0: cr1 - r0, 1:65] = xb.reshape(2, 128, 64, 64)[:, :, cr0:cr1, :]
        cyv = (base + 2 * j[None, :, None] + (p // 64)[:, None, None]
               + (t // 3)[None, None, :] - 1 + SH).astype(np.float32)
        in_maps.append({
            "xblk": xblk_cache[bb], "xconv": xc, "wofft": wofft_, "wt": wt_,
            "cy": cyv, "cx": cxv, "bvec": bvec_, "gamT": gamT_, "betT": betT_,
            "bT": bT_, "boff": boff_,
        })
    return in_maps


def kernel(x, w_off, b_off, w, b, gamma, beta):
    nc = get_nc(with_collective=True)
    in_maps = make_in_maps(x, w_off, b_off, w, b, gamma, beta)
    res = run_bass_kernel_spmd(nc, in_maps, core_ids=list(range(N_CORES)))
    out = np.empty((B, C, H, W), np.float32)
    for core in range(N_CORES):
        bb, half = core // 2, core % 2
        yo = res.results[core]["yout"]               # [2, 128, 2048]
        out[bb, :, 32 * half:32 * half + 32, :] = yo.reshape(256, 32, 64)
    return out


# revision 5
# speedup vs baseline: 1.1675x; 1.1675x over previous
"""Deformable conv v3: block-table gather + DVE/ACT split combine + DMA transpose.

Sharding: (batch=4) x (row-halves=2) -> 8 cores, 2048 positions/core.
Key changes vs baseline:
  - 2x2-corner block table xblk[4225, 1024]: ONE gather descriptor per
    (pos, tap) fetches all 4 bilinear corners (desc-count-bound gather).
  - corner combine: DVE does corners 0,1 (ts-mult + stt), ACT does 2,3
    (scale-copies), DVE wide tt-adds merge.
  - cols transposed to ch-major via batched DMA transpose on sync engine
    (out[c,j,p] = in[p,j,c]) -- no PE transposes, no psum copies.
  - main matmul accumulates per-tap into all 8 PSUM banks.
  - GN group reduce/broadcast via partition-crossing DMAs (no PSUM needed);
    one merged [16,4] pair-AllReduce.
"""
import sys

sys.path.insert(0, "/opt/trn_rl_repo/concourse")
sys.path.insert(0, "/opt/trn_rl_repo")

import numpy as np
import ml_dtypes

import concourse.bass as bass
import concourse.bacc as bacc
import concourse.tile as tile
import concourse.mybir as mybir
from concourse.bass_utils import run_bass_kernel_spmd

F32 = mybir.dt.float32
BF16 = mybir.dt.bfloat16
I16 = mybir.dt.int16
I32 = mybir.dt.int32
AOT = mybir.AluOpType
ACTF = mybir.ActivationFunctionType

B, C, H, W = 4, 256, 64, 64
P9 = 9
EPS = 1e-5
N_CORES = 8
NJ = 16
SH = 16.0


def _ident(nc, name, dt):
    from concourse.masks import make_identity
    t = nc.alloc_sbuf_tensor(name, [128, 128], dt)
    make_identity(nc, t.ap())
    return t.ap()


def build_nc(with_collective=True):
    nc = bacc.Bacc("TRN2", target_bir_lowering=False, debug=False,
                   num_devices=N_CORES if with_collective else 1)
    DIV = 32768.0 if with_collective else 16384.0

    # ---------------- I/O ----------------
    xblk = nc.dram_tensor("xblk", [4225, 1024], BF16, kind="ExternalInput")
    xconv = nc.dram_tensor("xconv", [2, 128, 34, 66], BF16, kind="ExternalInput").ap()
    wofft = nc.dram_tensor("wofft", [128, 2, 9, 18], BF16, kind="ExternalInput").ap()
    wt = nc.dram_tensor("wt", [128, 2, 9, 256], BF16, kind="ExternalInput").ap()
    cy = nc.dram_tensor("cy", [128, 16, 9], F32, kind="ExternalInput").ap()
    cx = nc.dram_tensor("cx", [128, 16, 9], F32, kind="ExternalInput").ap()
    bvec = nc.dram_tensor("bvec", [128, 2], F32, kind="ExternalInput").ap()
    gamT = nc.dram_tensor("gamT", [16, 8, 2], F32, kind="ExternalInput").ap()
    betT = nc.dram_tensor("betT", [16, 8, 2], F32, kind="ExternalInput").ap()
    bT = nc.dram_tensor("bT", [16, 8, 2], F32, kind="ExternalInput").ap()
    boff = nc.dram_tensor("boff", [18, 1], F32, kind="ExternalInput").ap()
    yout = nc.dram_tensor("yout", [2, 128, 2048], F32, kind="ExternalOutput").ap()

    with tile.TileContext(nc) as tc:
        # persistent sbuf tensors
        sb_wofft = nc.alloc_sbuf_tensor("sb_wofft", [128, 2, 9, 18], BF16).ap()
        sb_wt = nc.alloc_sbuf_tensor("sb_wt", [128, 2, 9, 256], BF16).ap()
        sb_cy = nc.alloc_sbuf_tensor("sb_cy", [128, 16, 9], F32).ap()
        sb_cx = nc.alloc_sbuf_tensor("sb_cx", [128, 16, 9], F32).ap()
        sb_bvec = nc.alloc_sbuf_tensor("sb_bvec", [128, 2], F32).ap()
        sb_gamT = nc.alloc_sbuf_tensor("sb_gamT", [16, 8, 2], F32).ap()
        sb_betT = nc.alloc_sbuf_tensor("sb_betT", [16, 8, 2], F32).ap()
        sb_bT = nc.alloc_sbuf_tensor("sb_bT", [16, 8, 2], F32).ap()
        sb_boff = nc.alloc_sbuf_tensor("sb_boff", [18, 1], F32).ap()
        offv = nc.alloc_sbuf_tensor("offv", [128, 16, 18], F32).ap()
        w4 = nc.alloc_sbuf_tensor("w4", [128, 16, 9, 4], F32).ap()
        idxs16 = nc.alloc_sbuf_tensor("idxs16", [128, 9, 128], I16)
        sums = nc.alloc_sbuf_tensor("sums", [128, 2, 4], F32).ap()
        sumsq = nc.alloc_sbuf_tensor("sumsq", [128, 2, 4], F32).ap()
        st2 = nc.alloc_sbuf_tensor("st2", [128, 2, 2], F32).ap()   # per o2
        st8 = nc.alloc_sbuf_tensor("st8", [16, 2, 8, 2], F32).ap()
        allst = nc.alloc_sbuf_tensor("allst", [16, 4], F32).ap()
        allst2 = nc.alloc_sbuf_tensor("allst2", [16, 4], F32).ap()
        mr = nc.alloc_sbuf_tensor("mr", [16, 2, 2], F32).ap()      # (mean,rstd) x o2
        scbn16 = nc.alloc_sbuf_tensor("scbn16", [16, 2, 8, 2], F32).ap()  # (sc,bn) x (i, o2)
        sc_pp = nc.alloc_sbuf_tensor("sc_pp", [128, 2], F32).ap()
        bn_pp = nc.alloc_sbuf_tensor("bn_pp", [128, 2], F32).ap()
        sb_eps = nc.alloc_sbuf_tensor("sb_eps", [16, 1], F32).ap()
        scr = nc.alloc_sbuf_tensor("scr", [128, 512], BF16).ap()
        scr2 = scr

        nc.vector.memset(sb_eps[:], EPS)
        id32 = _ident(nc, "id32", F32)

        # ---------------- loads ----------------
        xblk_ap = bass.AP(tensor=xblk, offset=0, ap=[[1024, 4225], [1, 1024]])

        with (
            tc.tile_pool(name="g", bufs=2) as gpool,
            tc.tile_pool(name="mth", bufs=18) as mth,
        ):
            sb_xconv = gpool.tile([128, 2, 34, 66], BF16, tag="g", name="sb_xconv")
            off_sb = gpool.tile([18, 2048], F32, tag="g", name="off_sb")
            nc.sync.dma_start(out=sb_xconv[:], in_=bass.AP(
                tensor=xconv.tensor, offset=0,
                ap=[[34 * 66, 128], [128 * 34 * 66, 2], [66, 34], [1, 66]]))
            nc.sync.dma_start(out=sb_wofft[:], in_=wofft)
            nc.sync.dma_start(out=sb_wt[:], in_=wt)
            nc.sync.dma_start(out=sb_cy[:], in_=cy)
            nc.sync.dma_start(out=sb_cx[:], in_=cx)
            nc.sync.dma_start(out=sb_bvec[:], in_=bvec)
            nc.sync.dma_start(out=sb_gamT[:], in_=gamT)
            nc.sync.dma_start(out=sb_betT[:], in_=betT)
            nc.sync.dma_start(out=sb_bT[:], in_=bT)
            nc.sync.dma_start(out=sb_boff[:], in_=boff)

            # ------------- phases 1-4, pipelined per position-half jh -------
            with tc.tile_pool(name="ps_pro", bufs=2, space="PSUM") as pss:
                _mtc = [0]

                def mt(shape=(128, 8, 9), dt=F32, tag="m"):
                    _mtc[0] += 1
                    return mth.tile(list(shape), dt, tag=tag, name=f"mt{_mtc[0]}",
                                    bufs=18 if tag == "m" else 4)

                for jh in range(2):
                    j8 = slice(jh * 8, (jh + 1) * 8)
                    # phase 1: offset conv for this half (rc chunks 2jh, 2jh+1)
                    for rc in (2 * jh, 2 * jh + 1):
                        ps = pss.tile([18, 512], F32, tag="ps", space="PSUM")
                        k = 0
                        for t in range(9):
                            for c2 in range(2):
                                ky, kx = t // 3, t % 3
                                rv = sb_xconv[:, c2, rc * 8 + ky: rc * 8 + ky + 8, kx: kx + 64]
                                nc.tensor.matmul(ps[:], lhsT=sb_wofft[:, c2, t, :], rhs=rv,
                                                 start=(k == 0), stop=(k == 17))
                                k += 1
                        nc.vector.tensor_scalar(out=off_sb[:, rc * 512:(rc + 1) * 512],
                                                in0=ps[:], scalar1=sb_boff[:, 0:1],
                                                scalar2=None, op0=AOT.add)
                    # phase 2: transpose offsets to [pos, 18]
                    for jj in range(jh * 8, (jh + 1) * 8):
                        pt = pss.tile([128, 18], F32, tag="ps", space="PSUM")
                        nc.tensor.transpose(pt[:], off_sb[:, jj * 128:(jj + 1) * 128],
                                            id32[:18, :18])
                        nc.vector.tensor_copy(out=offv[:, jj, :], in_=pt[:])

                    # phase 3: coords / weights / indices for this half
                    dy = offv[:, j8, 0:18:2]
                    dx = offv[:, j8, 1:18:2]
                    ys, xs = mt(), mt()
                    nc.vector.tensor_add(out=ys[:], in0=dy, in1=sb_cy[:, j8, :])
                    nc.vector.tensor_add(out=xs[:], in0=dx, in1=sb_cx[:, j8, :])
                    fy, fx, y0, x0 = mt(), mt(), mt(), mt()
                    # floor(v): i=round(v); floor = i - (i > v); frac = v - floor
                    for src_, fl_, fr_ in ((ys, y0, fy), (xs, x0, fx)):
                        ic = mt(dt=I32, tag="mi")
                        icf = mt()
                        gt_ = mt()
                        nc.vector.tensor_copy(out=ic[:], in_=src_[:])
                        nc.vector.tensor_copy(out=icf[:], in_=ic[:])
                        nc.vector.tensor_tensor(out=gt_[:], in0=icf[:], in1=src_[:], op=AOT.is_gt)
                        nc.vector.tensor_tensor(out=fl_[:], in0=icf[:], in1=gt_[:], op=AOT.subtract)
                        nc.vector.tensor_tensor(out=fr_[:], in0=src_[:], in1=fl_[:], op=AOT.subtract)
                    wy0, wx0 = mt(), mt()
                    nc.vector.tensor_scalar(out=wy0[:], in0=fy[:], scalar1=-1.0, scalar2=1.0,
                                            op0=AOT.mult, op1=AOT.add)
                    nc.vector.tensor_scalar(out=wx0[:], in0=fx[:], scalar1=-1.0, scalar2=1.0,
                                            op0=AOT.mult, op1=AOT.add)

                    def mask_in(src, lo, hi):
                        g_, l_, m_ = mt(), mt(), mt()
                        nc.vector.tensor_scalar(out=g_[:], in0=src[:], scalar1=lo, scalar2=None, op0=AOT.is_ge)
                        nc.vector.tensor_scalar(out=l_[:], in0=src[:], scalar1=hi, scalar2=None, op0=AOT.is_le)
                        nc.vector.tensor_tensor(out=m_[:], in0=g_[:], in1=l_[:], op=AOT.mult)
                        return m_

                    my0 = mask_in(y0, 16.0, 79.0)
                    my1 = mask_in(y0, 15.0, 78.0)
                    mx0 = mask_in(x0, 16.0, 79.0)
                    mx1 = mask_in(x0, 15.0, 78.0)

                    wy0e, wy1e, wx0e, wx1e = mt(), mt(), mt(), mt()
                    nc.vector.tensor_tensor(out=wy0e[:], in0=wy0[:], in1=my0[:], op=AOT.mult)
                    nc.vector.tensor_tensor(out=wy1e[:], in0=fy[:], in1=my1[:], op=AOT.mult)
                    nc.vector.tensor_tensor(out=wx0e[:], in0=wx0[:], in1=mx0[:], op=AOT.mult)
                    nc.vector.tensor_tensor(out=wx1e[:], in0=fx[:], in1=mx1[:], op=AOT.mult)

                    # w4 block-slot order: (y0,x0), (y0,x1), (y1,x0), (y1,x1)
                    nc.vector.tensor_tensor(out=w4[:, j8, :, 0], in0=wy0e[:], in1=wx0e[:], op=AOT.mult)
                    nc.vector.tensor_tensor(out=w4[:, j8, :, 1], in0=wy0e[:], in1=wx1e[:], op=AOT.mult)
                    nc.vector.tensor_tensor(out=w4[:, j8, :, 2], in0=wy1e[:], in1=wx0e[:], op=AOT.mult)
                    nc.vector.tensor_tensor(out=w4[:, j8, :, 3], in0=wy1e[:], in1=wx1e[:], op=AOT.mult)

                    # block idx: iy = clamp(y0s,15,79); idx = iy*65 + ix - 990
                    yc, xc, idxf = mt(), mt(), mt()
                    nc.vector.tensor_scalar(out=yc[:], in0=y0[:], scalar1=15.0, scalar2=79.0,
                                            op0=AOT.max, op1=AOT.min)
                    nc.vector.tensor_scalar(out=xc[:], in0=x0[:], scalar1=15.0, scalar2=79.0,
                                            op0=AOT.max, op1=AOT.min)
                    nc.vector.scalar_tensor_tensor(out=idxf[:], in0=yc[:], scalar=65.0,
                                                   in1=xc[:], op0=AOT.mult, op1=AOT.add)

                    # phase 4: idx transposes into wrapped layout
                    # idxs16[r, t, jj*8+q] = idxf[16q+r, jj-jh*8, t] - 990
                    idxt_t = mt(shape=(9, 8, 128), tag="it")
                    for jl in range(8):
                        pi = pss.tile([9, 128], F32, tag="ps", space="PSUM")
                        nc.tensor.transpose(pi[:], idxf[:, jl, :], id32[:, :])
                        nc.vector.tensor_copy(out=idxt_t[:, jl, :], in_=pi[:])
                    for jl in range(8):
                        jj = jh * 8 + jl
                        ptw = pss.tile([16, 8, 9], F32, tag="ps", space="PSUM")
                        for q in range(8):
                            nc.tensor.transpose(ptw[:, q, :],
                                                idxt_t[:, jl, 16 * q:16 * q + 16],
                                                id32[:9, :9])
                        nc.vector.tensor_scalar(
                            out=idxs16.ap()[0:16, :, jj * 8:(jj + 1) * 8],
                            in0=ptw[:].rearrange("r a t -> r t a"),
                            scalar1=-990.0, scalar2=None, op0=AOT.add)
                    for g8 in range(1, 8):
                        nc.sync.dma_start(
                            out=idxs16.ap()[g8 * 16:(g8 + 1) * 16, :, jh * 64:(jh + 1) * 64],
                            in_=idxs16.ap()[0:16, :, jh * 64:(jh + 1) * 64])

            # ---------------- phases 5-8: gather/combine/transpose/matmul ----
            with (
                tc.tile_pool(name="gb", bufs=6) as gbpool,
                tc.tile_pool(name="ac", bufs=4) as acpool,
                tc.tile_pool(name="tm", bufs=2) as tmpool,
                tc.tile_pool(name="rh", bufs=4) as rhpool,
                tc.tile_pool(name="psy", bufs=8, space="PSUM") as psy,
                tc.tile_pool(name="yo", bufs=2) as yopool,
                tc.tile_pool(name="dram", bufs=2, space="DRAM") as drpool,
            ):
                ps_y = []
                for o2 in range(2):
                    for nt in range(4):
                        ps_y.append(psy.tile([128, 512], F32, tag="y", space="PSUM",
                                             name=f"psy{o2}_{nt}"))
                gsems = [nc.alloc_semaphore(f"gsem{i}") for i in range(18)]
                if with_collective:
                    win_ = drpool.tile([16, 4], F32, tag="win")
                    wout = drpool.tile([16, 4], F32, tag="wout")
                    nc.sync.dma_start(out=win_[:], in_=allst[:])
                    nc.gpsimd.collective_compute(
                        "AllReduce", AOT.add,
                        replica_groups=[[0, 1], [2, 3], [4, 5], [6, 7]],
                        ins=[win_.opt()], outs=[wout.opt()])

                for t in range(9):
                    for jh in range(2):
                        g_ = gbpool.tile([128, 8, 1024], BF16, tag="gb")
                        nc.gpsimd.dma_gather(
                            out_ap=g_[:], in_ap=xblk_ap,
                            idxs_ap=idxs16.ap()[:, t, jh * 64:(jh + 1) * 64],
                            num_idxs=1024, num_idxs_reg=1024,
                            elem_size=1024, elem_step=1024,
                            prepare_only=True, sem=gsems[2 * t + jh])
                        nc.gpsimd.trigger_dma(count=1)
                        acc = acpool.tile([128, 2, 8, 128], BF16, tag="ac")
                        rhs_t = rhpool.tile([128, 2, 8, 128], BF16, tag="rh")
                        tA = tmpool.tile([128, 2, 2, 8, 128], BF16, tag="tm")
                        nc.vector.wait_ge(gsems[2 * t + jh], 16)
                        nc.scalar.wait_ge(gsems[2 * t + jh], 16)
                        for j in range(8):
                            jj = jh * 8 + j
                            gv = [g_[:, j, k * 256:(k + 1) * 256]
                                  .rearrange("p (a b) -> p a b", a=2) for k in range(4)]
                            sc = [w4[:, jj, t, k:k + 1] for k in range(4)]
                            # DVE: corners 0,1
                            nc.vector.tensor_scalar(out=acc[:, :, j, :], in0=gv[0],
                                                    scalar1=sc[0], scalar2=None, op0=AOT.mult)
                            nc.vector.scalar_tensor_tensor(out=acc[:, :, j, :], in0=gv[1],
                                                           scalar=sc[1], in1=acc[:, :, j, :],
                                                           op0=AOT.mult, op1=AOT.add)
                            # ACT: corners 2,3
                            nc.scalar.activation(out=tA[:, 0, :, j, :], in_=gv[2],
                                                 func=ACTF.Copy, scale=sc[2])
                            nc.scalar.activation(out=tA[:, 1, :, j, :], in_=gv[3],
                                                 func=ACTF.Copy, scale=sc[3])
                        # DVE wide adds
                        nc.vector.tensor_tensor(out=tA[:, 0], in0=tA[:, 0], in1=tA[:, 1],
                                                op=AOT.add)
                        nc.vector.tensor_tensor(out=acc[:], in0=acc[:], in1=tA[:, 0],
                                                op=AOT.add)
                        # DMA transpose to ch-major: rhs_t[c, j, p] = acc[p, j, c]
                        for c2 in range(2):
                            nc.sync.dma_start_transpose(
                                out=rhs_t[:, c2, :, :],
                                in_=acc[:, c2, :, :])
                        # main matmul accumulation for this half
                        for o2 in range(2):
                            for c2 in range(2):
                                lt = sb_wt[:, c2, t, o2 * 128:(o2 + 1) * 128]
                                for nl in range(2):
                                    nt = 2 * jh + nl
                                    nc.tensor.matmul(
                                        ps_y[o2 * 4 + nt][:], lhsT=lt,
                                        rhs=rhs_t[:, c2, nl * 4:(nl + 1) * 4, :]
                                        .rearrange("c a b -> c (a b)"),
                                        start=(t == 0 and c2 == 0),
                                        stop=(t == 8 and c2 == 1))
                        if t == 8:
                            for o2 in range(2):
                                for nl in range(2):
                                    nt = 2 * jh + nl
                                    nc.scalar.activation(
                                        out=scr[:], in_=ps_y[o2 * 4 + nt][:],
                                        func=ACTF.Copy,
                                        accum_out=sums[:, o2, nt:nt + 1])
                                    nc.scalar.activation(
                                        out=scr2[:], in_=ps_y[o2 * 4 + nt][:],
                                        func=ACTF.Square,
                                        accum_out=sumsq[:, o2, nt:nt + 1])

                # ---------------- phase 9: GN stats ----------------
                for o2 in range(2):
                    sy = mth.tile([128, 1], F32, tag="s1", bufs=12, name="sy")
                    qy = mth.tile([128, 1], F32, tag="s1", bufs=12)
                    t1_ = mth.tile([128, 1], F32, tag="s1", bufs=12)
                    t2_ = mth.tile([128, 1], F32, tag="s1", bufs=12)
                    t3_ = mth.tile([128, 1], F32, tag="s1", bufs=12)
                    nc.vector.reduce_sum(out=sy[:], in_=sums[:, o2, :],
                                         axis=mybir.AxisListType.X)
                    nc.vector.reduce_sum(out=qy[:], in_=sumsq[:, o2, :],
                                         axis=mybir.AxisListType.X)
                    b_ap = sb_bvec[:, o2:o2 + 1]
                    # Sz = Sy + 2048*b ; Qz = Qy + 2*b*Sy + 2048*b^2
                    nc.vector.scalar_tensor_tensor(out=st2[:, o2, 0:1], in0=b_ap,
                                                   scalar=2048.0, in1=sy[:],
                                                   op0=AOT.mult, op1=AOT.add)
                    nc.vector.scalar_tensor_tensor(out=t1_[:], in0=b_ap, scalar=2.0,
                                                   in1=sy[:], op0=AOT.mult, op1=AOT.mult)
                    nc.vector.scalar_tensor_tensor(out=t2_[:], in0=b_ap, scalar=2048.0,
                                                   in1=b_ap, op0=AOT.mult, op1=AOT.mult)
                    nc.vector.tensor_tensor(out=t3_[:], in0=t1_[:], in1=t2_[:], op=AOT.add)
                    nc.vector.tensor_tensor(out=st2[:, o2, 1:2], in0=qy[:], in1=t3_[:], op=AOT.add)
                    # partition fold 128 -> 16 groups x 8: st8[g, o2, i, c] = st2[8g+i, o2, c]
                    nc.sync.dma_start(out=st8[:, o2, :, :], in_=st2[:, o2, :])
                # tree-add over i: [16,2,8,2] -> allst [16,4]
                a4 = mth.tile([16, 2, 4, 2], F32, tag="a4", bufs=2)
                a2 = mth.tile([16, 2, 2, 2], F32, tag="a2", bufs=2)
                nc.vector.tensor_tensor(out=a4[:], in0=st8[:, :, 0:4, :],
                                        in1=st8[:, :, 4:8, :], op=AOT.add)
                nc.vector.tensor_tensor(out=a2[:], in0=a4[:, :, 0:2, :],
                                        in1=a4[:, :, 2:4, :], op=AOT.add)
                nc.vector.tensor_tensor(out=allst[:].rearrange("p (a b) -> p a b", a=2),
                                        in0=a2[:, :, 0, :], in1=a2[:, :, 1, :], op=AOT.add)

                # ---------------- phase 10: pair AllReduce ----------------
                if with_collective:
                    bin_ = drpool.tile([16, 4], F32, tag="cin")
                    bout = drpool.tile([16, 4], F32, tag="cout")
                    nc.sync.dma_start(out=bin_[:], in_=allst[:])
                    nc.gpsimd.collective_compute(
                        "AllReduce", AOT.add,
                        replica_groups=[[0, 1], [2, 3], [4, 5], [6, 7]],
                        ins=[bin_.opt()], outs=[bout.opt()])
                    nc.sync.dma_start(out=allst2[:], in_=bout[:])
                else:
                    nc.vector.tensor_copy(out=allst2[:], in_=allst[:])

                # ---------------- phase 11: mean/rstd, scale/bias, relu ------
                var = mth.tile([16, 2], F32, tag="v16", bufs=2)
                m2 = mth.tile([16, 2], F32, tag="v16", bufs=2)
                av2 = allst2.rearrange("p (a b) -> p a b", a=2)
                nc.vector.tensor_scalar(out=mr[:, :, 0], in0=av2[:, :, 0],
                                        scalar1=1.0 / DIV, scalar2=None, op0=AOT.mult)
                nc.vector.tensor_tensor(out=m2[:], in0=mr[:, :, 0], in1=mr[:, :, 0],
                                        op=AOT.mult)
                nc.vector.tensor_scalar(out=var[:], in0=av2[:, :, 1],
                                        scalar1=1.0 / DIV, scalar2=None, op0=AOT.mult)
                nc.vector.tensor_tensor(out=var[:], in0=var[:], in1=m2[:], op=AOT.subtract)
                nc.scalar.activation(out=var[:], in_=var[:], func=ACTF.Sqrt, bias=sb_eps[:])
                nc.vector.reciprocal(out=mr[:, :, 1], in_=var[:])
                # sc16 = gamT * rstd ; bn16 = (bT - mean)*sc16 + betT
                tb = mth.tile([16, 8, 2], F32, tag="tb", bufs=4)
                for o2 in range(2):
                    nc.vector.tensor_scalar(out=scbn16[:, 0, :, o2], in0=sb_gamT[:, :, o2],
                                            scalar1=mr[:, o2, 1:2], scalar2=None, op0=AOT.mult)
                    nc.vector.tensor_scalar(out=tb[:, :, o2], in0=sb_bT[:, :, o2],
                                            scalar1=mr[:, o2, 0:1], scalar2=None,
                                            op0=AOT.subtract)
                nc.vector.tensor_tensor(out=tb[:], in0=tb[:], in1=scbn16[:, 0], op=AOT.mult)
                nc.vector.tensor_tensor(out=scbn16[:, 1], in0=tb[:], in1=sb_betT[:], op=AOT.add)
                # partition-expand [16, 2, 8] -> [128, 2]: sc_pp[8g+i, o2]
                nc.sync.dma_start(out=sc_pp[:], in_=scbn16[:, 0])
                nc.sync.dma_start(out=bn_pp[:], in_=scbn16[:, 1])

                for nt in range(4):
                    for o2 in range(2):
                        yo = yopool.tile([128, 512], F32, tag="yo")
                        if (nt + o2) % 2 == 0:
                            nc.scalar.activation(out=yo[:], in_=ps_y[o2 * 4 + nt][:],
                                                 func=ACTF.Relu, scale=sc_pp[:, o2:o2 + 1],
                                                 bias=bn_pp[:, o2:o2 + 1])
                        else:
                            nc.vector.tensor_scalar(out=yo[:], in0=ps_y[o2 * 4 + nt][:],
                                                    scalar1=sc_pp[:, o2:o2 + 1],
                                                    scalar2=bn_pp[:, o2:o2 + 1],
                                                    op0=AOT.mult, op1=AOT.add)
                            nc.vector.tensor_scalar(out=yo[:], in0=yo[:],
                                                    scalar1=0.0, scalar2=None,
                                                    op0=AOT.max)
                        nc.sync.dma_start(out=yout[o2, :, nt * 512:(nt + 1) * 512], in_=yo[:])

    nc.compile()
    return nc


# ---------------------------------------------------------------------------
# host side
# ---------------------------------------------------------------------------
_NC_CACHE = {}


def get_nc(with_collective=True):
    key = with_collective
    if key not in _NC_CACHE:
        _NC_CACHE[key] = build_nc(with_collective)
    return _NC_CACHE[key]


def make_in_maps(x, w_off, b_off, w, b, gamma, beta):
    p = np.arange(128)
    j = np.arange(16)
    t = np.arange(9)
    cxv = (((p % 64)[:, None, None] + (t % 3)[None, None, :] - 1 + SH)
           + np.zeros((1, 16, 1))).astype(np.float32)
    # weight layouts
    w4d = w.reshape(256, 2, 128, 3, 3)
    wt_ = np.ascontiguousarray(
        w4d.reshape(256, 2, 128, 9).transpose(2, 1, 3, 0)).astype(ml_dtypes.bfloat16)
    wo4d = w_off.reshape(18, 2, 128, 9)
    wofft_ = np.ascontiguousarray(wo4d.transpose(2, 1, 3, 0)).astype(ml_dtypes.bfloat16)
    bvec_ = np.ascontiguousarray(b.reshape(2, 128).T).astype(np.float32)
    boff_ = b_off.reshape(18, 1).astype(np.float32)
    # [16 g, 2 o2, 8 i] layouts: value at channel o2*128 + 8g + i
    gamT_ = np.ascontiguousarray(gamma.reshape(2, 16, 8).transpose(1, 2, 0)).astype(np.float32)
    betT_ = np.ascontiguousarray(beta.reshape(2, 16, 8).transpose(1, 2, 0)).astype(np.float32)
    bT_ = np.ascontiguousarray(b.reshape(2, 16, 8).transpose(1, 2, 0)).astype(np.float32)

    in_maps = []
    xblk_cache = {}
    for core in range(N_CORES):
        bb, half = core // 2, core % 2
        base = 32 * half
        xb = x[bb]                                   # [256, 64, 64]
        if bb not in xblk_cache:
            xp = np.zeros((66, 66, 256), np.float32)
            xp[1:65, 1:65, :] = xb.transpose(1, 2, 0)
            blk = np.stack([xp[0:65, 0:65], xp[0:65, 1:66],
                            xp[1:66, 0:65], xp[1:66, 1:66]], axis=2)  # [65,65,4,256]
            xblk_cache[bb] = np.ascontiguousarray(
                blk.reshape(4225, 1024)).astype(ml_dtypes.bfloat16)
        xc = np.zeros((2, 128, 34, 66), ml_dtypes.bfloat16)
        r0, r1 = base - 1, base + 33
        cr0, cr1 = max(r0, 0), min(r1, 64)
        xc[:, :, cr0 - r0: cr1 - r0, 1:65] = xb.reshape(2, 128, 64, 64)[:, :, cr0:cr1, :]
        cyv = (base + 2 * j[None, :, None] + (p // 64)[:, None, None]
               + (t // 3)[None, None, :] - 1 + SH).astype(np.float32)
        in_maps.append({
            "xblk": xblk_cache[bb], "xconv": xc, "wofft": wofft_, "wt": wt_,
            "cy": cyv, "cx": cxv, "bvec": bvec_, "gamT": gamT_, "betT": betT_,
            "bT": bT_, "boff": boff_,
        })
    return in_maps


def kernel(x, w_off, b_off, w, b, gamma, beta):
    nc = get_nc(with_collective=True)
    in_maps = make_in_maps(x, w_off, b_off, w, b, gamma, beta)
    res = run_bass_kernel_spmd(nc, in_maps, core_ids=list(range(N_CORES)))
    out = np.empty((B, C, H, W), np.float32)
    for core in range(N_CORES):
        bb, half = core // 2, core % 2
        yo = res.results[core]["yout"]               # [2, 128, 2048]
        out[bb, :, 32 * half:32 * half + 32, :] = yo.reshape(256, 32, 64)
    return out
